# revision 20
# baseline (speedup 1.0000x reference)
"""Trainium2 Bass kernel for nn_CrossAttention (B=8, N=M=1024, D=1024, DK=768, H=16).

Sharding: data-parallel over batch B=8 -> one batch item per NeuronCore.
No collectives; attn.mean(dim=1) is over heads, all heads of a batch item
live on one core.

v3 design:
  - bf16 projections (host-converted inputs/weights), f16 attention.
  - Fill: K proj + V proj (staged inputs, few big DMAs) -> Q proj pairs
    0/1 + QK_0 borrowing the QK psum ring.
  - Steady-state window p: PE does PV(p-1) first (starts the softmax-sum
    drain early), then QK(p) head-A groups, then head-B groups, with
    Qproj(p+2) spliced in; scalar does the 16 [128,1024] exp ACTs; DVE
    does reciprocal + the mean accumulation; gpsimd does the fused
    outcat evac+normalize straight out of PSUM.
  - Head-A exp tiles are fully drained by the mean before head-B ones,
    so the next window's exp ACTs only WAR-wait on the early mean ops.
  - rbc reciprocal rows are broadcast by DMA (no engine time) into
    [128, 2048] tiles (2 copies -> fewer AP dim switches in the mean).
  - O-projection runs c-outer over 8 psum banks freed by QK/PV, with
    w_o column-slices streamed by DMA, overlapping the final mean.
  - Outputs stored f16, host upcasts.
"""

import sys
import types

sys.path.insert(0, "/opt/trn_rl_repo")
sys.path.insert(0, "/root/.axon_site")

import numpy as np

N_CORES = 8
B, N, M = 8, 1024, 1024
D = 1024      # Q_DIM
DK = 768      # K_DIM
H = 16        # heads
HD = 64       # head dim
SCALE = HD ** -0.5  # 0.125
NP = 8        # head pairs


def _install_ntff_hook():
    """Make trace=True work under axon (antenv.axon_hooks shim)."""
    if "antenv.axon_hooks" in sys.modules:
        return
    try:
        import antenv
        hooks_mod = types.ModuleType("antenv.axon_hooks")
        _hook = [None]
        hooks_mod.set_axon_ntff_profile_hook = lambda h: _hook.__setitem__(0, h)
        hooks_mod.get_axon_ntff_profile_hook = lambda: _hook[0]
        sys.modules["antenv.axon_hooks"] = hooks_mod
        antenv.axon_hooks = hooks_mod
        from trn_agent_boot.trn_boot import _ntff_profile_via_ctypes
        hooks_mod.set_axon_ntff_profile_hook(
            _ntff_profile_via_ctypes("/opt/axon/libaxon_pjrt.so")
        )
    except Exception:
        pass


_CACHE = {}


def build_module():
    if "nc" in _CACHE:
        return _CACHE["nc"]

    import concourse.tile as tile
    import concourse.mybir as mybir
    from concourse import bacc, library_config

    f32 = mybir.dt.float32
    bf16 = mybir.dt.bfloat16
    f16 = mybir.dt.float16
    AF = mybir.ActivationFunctionType

    nc = bacc.Bacc("TRN2", target_bir_lowering=False, debug=False,
                   num_devices=N_CORES)

    # ---- DRAM tensors (per-core shard) ----
    d_qT = nc.dram_tensor("qT_in", [D, N], bf16, kind="ExternalInput").ap()
    d_kT = nc.dram_tensor("kT_in", [DK, M], bf16, kind="ExternalInput").ap()
    d_vT = nc.dram_tensor("vT_in", [DK, M], bf16, kind="ExternalInput").ap()
    d_wqT = nc.dram_tensor("wqT", [D, D], bf16, kind="ExternalInput").ap()
    d_wkT = nc.dram_tensor("wkT", [DK, D], bf16, kind="ExternalInput").ap()
    d_wvT = nc.dram_tensor("wvT", [DK, D], bf16, kind="ExternalInput").ap()
    d_woT = nc.dram_tensor("woT16", [D, D], f16, kind="ExternalInput").ap()
    d_bq = nc.dram_tensor("bq_pp", [128, 8], f32, kind="ExternalInput").ap()
    d_bk = nc.dram_tensor("bk_pp", [128, 8], f32, kind="ExternalInput").ap()
    d_bv = nc.dram_tensor("bv_r", [1, D], bf16, kind="ExternalInput").ap()
    d_bo = nc.dram_tensor("bo_r", [1, D], bf16, kind="ExternalInput").ap()
    d_ones = nc.dram_tensor("ones_r", [1, 512], bf16, kind="ExternalInput").ap()
    d_out = nc.dram_tensor("out", [N, D], f16, kind="ExternalOutput").ap()
    d_amT = nc.dram_tensor("attn_meanT", [M, N], f16, kind="ExternalOutput").ap()

    with tile.TileContext(nc) as tc:
        nc.gpsimd.load_library(library_config.proxy)

        # ---------------- pools (stack order) -----------------------------
        const = tc.alloc_tile_pool(name="const", bufs=1)
        persist = tc.alloc_tile_pool(name="persist", bufs=1)
        xqP = tc.alloc_tile_pool(name="xqP", bufs=1)
        wqsp = tc.alloc_tile_pool(name="wqsp", bufs=2)
        wost = tc.alloc_tile_pool(name="wost", bufs=2)
        ostp = tc.alloc_tile_pool(name="ostp", bufs=2)

        ones_row = const.tile([1, 512], bf16, tag="ones", name="ones")
        nc.sync.dma_start(ones_row[:], d_ones[:, :])
        bq_sb = const.tile([128, 8], f32, tag="bq", name="bq")
        bk_sb = const.tile([128, 8], f32, tag="bk", name="bk")
        bv_sb = const.tile([1, D], bf16, tag="bv", name="bv")
        bo_sb = const.tile([1, D], bf16, tag="bo", name="bo")
        nc.sync.dma_start(bq_sb[:], d_bq[:, :])
        nc.sync.dma_start(bk_sb[:], d_bk[:, :])
        nc.sync.dma_start(bv_sb[:], d_bv[:, :])
        nc.sync.dma_start(bo_sb[:], d_bo[:, :])

        kT = [persist.tile([128, M], f16, tag=f"kT{j}", name=f"kT{j}")
              for j in range(NP)]
        qT = [persist.tile([128, N], f16, tag=f"qT{j}", name=f"qT{j}")
              for j in range(3)]
        v_sb = [persist.tile([128, H, HD + 1], f16, tag=f"v{j}", name=f"v{j}")
                for j in range(8)]
        outcat = [persist.tile([128, N], f16, tag=f"oc{c}", name=f"oc{c}")
                  for c in range(8)]
        acc = persist.tile([128, 8 * N], f16, tag="acc", name="acc")
        tmp = persist.tile([128, 4 * N], f16, tag="tmp", name="tmp")

        for mj in range(8):
            nc.vector.memset(v_sb[mj][:, :, HD:HD + 1], 1.0)

        # ---------------- fill: K proj + V proj ---------------------------
        xkvp = tc.alloc_tile_pool(name="xkvp", bufs=1)
        xkbig = xkvp.tile([128, 6 * 1024], bf16, tag="xk", name="xk")
        xvbig = xkvp.tile([128, 6 * 1024], bf16, tag="xv", name="xv")
        wvbig = xkvp.tile([128, 6 * 1024], bf16, tag="wv", name="wv")
        nc.sync.dma_start(
            xkbig[:].rearrange("p (a b) -> p a b", a=6),
            d_kT[:, :].rearrange("(a p) b -> p a b", p=128))
        nc.sync.dma_start(
            xvbig[:].rearrange("p (a b) -> p a b", a=6),
            d_vT[:, :].rearrange("(a p) b -> p a b", p=128))
        nc.sync.dma_start(
            wvbig[:].rearrange("p (a b) -> p a b", a=6),
            d_wvT[:, :].rearrange("(a p) b -> p a b", p=128))
        xqbig = xqP.tile([128, 8 * 1024], bf16, tag="xq", name="xq")
        nc.sync.dma_start(
            xqbig[:].rearrange("p (a b) -> p a b", a=8),
            d_qT[:, :].rearrange("(a p) b -> p a b", p=128))
        wksp = tc.alloc_tile_pool(name="wksp", bufs=2)

        projps = tc.alloc_tile_pool(name="projps", bufs=2, space="PSUM")

        # K projection: kT[j][d_local, m]
        for j in range(NP):
            wks = wksp.tile([128, 6 * 128], bf16, tag="wks", name="wks")
            nc.sync.dma_start(
                wks[:].rearrange("p (a b) -> p a b", a=6),
                d_wkT[:, j * 128:(j + 1) * 128].rearrange(
                    "(a p) b -> p a b", p=128))
            for nb in range(2):
                ps = projps.tile([128, 512], f32, tag="pj", name="pj")
                for c in range(6):
                    nc.tensor.matmul(
                        ps[:], wks[:, c * 128:(c + 1) * 128],
                        xkbig[:, c * 1024 + nb * 512:c * 1024 + nb * 512 + 512],
                        start=(c == 0), stop=(c == 5))
                nc.scalar.activation(
                    kT[j][:, nb * 512:(nb + 1) * 512], ps[:],
                    AF.Identity, bias=bk_sb[:, j:j + 1], scale=1.0)

        # V projection (bulk): v_sb[mj][:, h, 0:64]
        for ob in range(2):
            for mj in range(8):
                ps = projps.tile([128, 512], f32, tag="pj", name="pj")
                for c in range(6):
                    nc.tensor.matmul(
                        ps[:],
                        xvbig[:, c * 1024 + mj * 128:c * 1024 + mj * 128 + 128],
                        wvbig[:, c * 1024 + ob * 512:c * 1024 + ob * 512 + 512],
                        start=(c == 0), stop=False)
                nc.tensor.matmul(
                    ps[:], ones_row[:, 0:128],
                    bv_sb[:, ob * 512:(ob + 1) * 512],
                    start=False, stop=True)
                nc.scalar.activation(
                    v_sb[mj][:, ob * 8:(ob + 1) * 8, 0:HD],
                    ps[:].rearrange("p (a b) -> p a b", a=8),
                    AF.Copy)

        projps.release()
        wksp.release()
        xkvp.release()

        # ---------------- attention pools ---------------------------------
        expp = tc.alloc_tile_pool(name="expp", bufs=2)
        rbcp = tc.alloc_tile_pool(name="rbcp", bufs=2)
        sump = tc.alloc_tile_pool(name="sump", bufs=1)
        qkps = tc.alloc_tile_pool(name="qkps", bufs=2, space="PSUM")
        pvps = tc.alloc_tile_pool(name="pvps", bufs=1, space="PSUM")

        def qk_tile():
            return qkps.tile([128, 1024], f32, tag="qk", name="qk")

        def qproj(p):
            """Q projection for pair p -> qT[p % 3], borrowing a QK tile."""
            wqs = wqsp.tile([128, 8 * 128], bf16, tag="wqs", name="wqs")
            nc.sync.dma_start(
                wqs[:].rearrange("p (a b) -> p a b", a=8),
                d_wqT[:, p * 128:(p + 1) * 128].rearrange(
                    "(a p) b -> p a b", p=128))
            ps = qk_tile()
            for c in range(8):
                for nb in range(2):
                    nc.tensor.matmul(
                        ps[:, nb * 512:(nb + 1) * 512],
                        wqs[:, c * 128:(c + 1) * 128],
                        xqbig[:, c * 1024 + nb * 512:c * 1024 + nb * 512 + 512],
                        start=(c == 0), stop=(c == 7))
            for nb in range(2):
                nc.scalar.activation(
                    qT[p % 3][:, nb * 512:(nb + 1) * 512],
                    ps[:, nb * 512:(nb + 1) * 512],
                    AF.Identity, bias=bq_sb[:, p:p + 1], scale=1.0)

        qproj(0)

        exps = {}
        pvts = {}

        for p in range(NP + 1):
            if p < NP:
                eA = expp.tile([128, 8 * N], f16, tag="eA", name="eA")
                eB = expp.tile([128, 8 * N], f16, tag="eB", name="eB")
                exps[p] = (eA, eB)
            if p >= 1:
                pvA, pvB = pvts[p - 1]
                cA, cB = exps[p - 1]

                # ---- PV(p-1) first: head A fully, then head B, so the
                # head-A drain chain can start at PV half-time ----
                for nb in range(2):
                    for mj in range(8):
                        col = mj * 1024 + nb * 512
                        nc.tensor.matmul(
                            pvA[:, nb * 512:(nb + 1) * 512],
                            v_sb[mj][:, 2 * (p - 1), :],
                            cA[:, col:col + 512],
                            start=(mj == 0), stop=(mj == 7))
                for nb in range(2):
                    for mj in range(8):
                        col = mj * 1024 + nb * 512
                        nc.tensor.matmul(
                            pvB[:, nb * 512:(nb + 1) * 512],
                            v_sb[mj][:, 2 * (p - 1) + 1, :],
                            cB[:, col:col + 512],
                            start=(mj == 0), stop=(mj == 7))

            # ---- drain part 1: per-head rowsum -> reciprocal -> rbc ----
            if p >= 1:
                q = p - 1
                sA = sump.tile([1, 1024], f32, tag="sA", name="sA")
                sB = sump.tile([1, 1024], f32, tag="sB", name="sB")
                r16A = sump.tile([1, 1024], f16, tag="r16A", name="r16A")
                r16B = sump.tile([1, 1024], f16, tag="r16B", name="r16B")
                rbcA = rbcp.tile([128, 1024], f16, tag="rA", name="rA")
                rbcB = rbcp.tile([128, 1024], f16, tag="rB", name="rB")
                nc.scalar.copy(sA[:], pvA[64:65, :])
                nc.vector.reciprocal_approx_fast(out=sA[:], in_=sA[:])
                nc.vector.tensor_scalar_mul(r16A[:], sA[:], 1.0 / H)
                nc.gpsimd.partition_broadcast(rbcA[:], r16A[:])
                nc.scalar.copy(sB[:], pvB[64:65, :])
                nc.vector.reciprocal_approx_fast(out=sB[:], in_=sB[:])
                nc.vector.tensor_scalar_mul(r16B[:], sB[:], 1.0 / H)
                nc.gpsimd.partition_broadcast(rbcB[:], r16B[:])

            def qk_groups(hb):
                et = exps[p][hb]
                r0, r1 = (0, 64) if hb == 0 else (64, 128)
                tp = (0, 0) if hb == 0 else (64, 0)
                for mj in range(8):
                    ps = qk_tile()
                    for nb in range(2):
                        nc.tensor.matmul(
                            ps[:, nb * 512:(nb + 1) * 512],
                            kT[p][r0:r1, mj * 128:(mj + 1) * 128],
                            qT[p % 3][r0:r1, nb * 512:(nb + 1) * 512],
                            start=True, stop=True, tile_position=tp)
                    nc.scalar.activation(
                        et[:, mj * 1024:(mj + 1) * 1024], ps[:],
                        AF.Exp, scale=SCALE)
                    # splice Qproj bursts into the QK stream
                    if hb == 0 and mj == 3 and 1 <= p + 1 < NP:
                        qproj(p + 1)

            # ---- QK(p) head A groups ----
            if p < NP:
                qk_groups(0)

            # ---- drain part 2: outcat + softmax-mean ----
            if p >= 1:
                nc.scalar.copy(outcat[q][0:64, :], pvA[0:64, :])
                nc.scalar.copy(outcat[q][64:128, :], pvB[0:64, :])
                nc.gpsimd.tensor_mul(outcat[q][0:64, :],
                                     outcat[q][0:64, :], rbcA[0:64, :])
                nc.gpsimd.tensor_mul(outcat[q][64:128, :],
                                     outcat[q][64:128, :],
                                     rbcB[64:128, :])

                # softmax-mean accumulation on DVE: head A fully first so
                # eA frees early (next window's exp-A ACTs wait on it)
                cA, cB = exps[q]
                rAv = rbcA[:, :].unsqueeze(1).broadcast_to([128, 4, 1024])
                rBv = rbcB[:, :].unsqueeze(1).broadcast_to([128, 4, 1024])
                tv = tmp[:, :].rearrange("p (a b) -> p a b", a=4)
                for hb in range(2):
                    ev = (cA, cB)[hb]
                    rv = (rAv, rBv)[hb]
                    for half in range(2):
                        sl = slice(half * 4096, (half + 1) * 4096)
                        e3 = ev[:, sl].rearrange("p (a b) -> p a b", a=4)
                        a3 = acc[:, sl].rearrange("p (a b) -> p a b", a=4)
                        if q == 0 and hb == 0:
                            nc.vector.tensor_mul(a3, e3, rv)
                        else:
                            nc.vector.tensor_mul(tv, e3, rv)
                            nc.vector.tensor_add(a3, a3, tv)

            # ---- QK(p) head B groups ----
            if p < NP:
                qk_groups(1)

            if p < NP:
                pvts[p] = (
                    pvps.tile([65, 1024], f32, tag="pvA", name="pvA"),
                    pvps.tile([65, 1024], f32, tag="pvB", name="pvB"))
            if p >= 1:
                del exps[q]
                del pvts[q]

        # ---------------- O-projection + outputs --------------------------
        pvps.release()
        qkps.release()
        oprojps = tc.alloc_tile_pool(name="oprojps", bufs=1, space="PSUM")
        ops_t = [oprojps.tile([128, 512], f32, tag=f"op{nj}", name=f"op{nj}")
                 for nj in range(8)]

        for ob in range(2):
            for c in range(8):
                woc = wost.tile([128, 512], f16, tag="woc", name="woc")
                nc.sync.dma_start(
                    woc[:], d_woT[c * 128:(c + 1) * 128,
                                  ob * 512:(ob + 1) * 512])
                for nj in range(8):
                    nc.tensor.matmul(
                        ops_t[nj][:],
                        outcat[c][:, nj * 128:(nj + 1) * 128],
                        woc[:],
                        start=(c == 0), stop=False)
            for nj in range(8):
                nc.tensor.matmul(
                    ops_t[nj][:],
                    ones_row[:, 0:128],
                    bo_sb[:, ob * 512:(ob + 1) * 512],
                    start=False, stop=True)
                ost = ostp.tile([128, 512], f16, tag="ost", name="ost")
                nc.scalar.activation(ost[:], ops_t[nj][:], AF.Copy)
                nc.sync.dma_start(
                    d_out[nj * 128:(nj + 1) * 128,
                          ob * 512:(ob + 1) * 512], ost[:])

        # attn_meanT straight from the f16 accumulator
        for mj in range(8):
            nc.sync.dma_start(d_amT[mj * 128:(mj + 1) * 128, :],
                              acc[:, mj * 1024:(mj + 1) * 1024])

        oprojps.release()
        sump.release()
        rbcp.release()
        expp.release()
        ostp.release()
        wost.release()
        wqsp.release()
        xqP.release()
        persist.release()
        const.release()

    nc.compile()
    _CACHE["nc"] = nc
    return nc


def prepare_in_maps(query, key, value, w_q, b_q, w_k, b_k, w_v, b_v, w_o, b_o):
    """Host-side sharding + layout prep. Returns list of per-core input dicts."""
    import ml_dtypes
    f = np.float32
    bf = ml_dtypes.bfloat16
    wqT = np.ascontiguousarray(np.asarray(w_q, f).T).astype(bf)
    wkT = np.ascontiguousarray(np.asarray(w_k, f).T).astype(bf)
    wvT = np.ascontiguousarray(np.asarray(w_v, f).T).astype(bf)
    woT16 = np.ascontiguousarray(
        (np.asarray(w_o, f).T * np.float32(H)).astype(np.float16))
    bq_pp = np.ascontiguousarray(np.asarray(b_q, f).reshape(8, 128).T)
    bk_pp = np.ascontiguousarray(np.asarray(b_k, f).reshape(8, 128).T)
    bv_r = np.asarray(b_v, f).reshape(1, D).astype(bf)
    bo_r = np.asarray(b_o, f).reshape(1, D).astype(bf)
    ones_r = np.ones((1, 512), bf)
    query = np.asarray(query, f)
    key = np.asarray(key, f)
    value = np.asarray(value, f)

    in_maps = []
    for b in range(B):
        in_maps.append({
            "qT_in": np.ascontiguousarray(query[b].T).astype(bf),
            "kT_in": np.ascontiguousarray(key[b].T).astype(bf),
            "vT_in": np.ascontiguousarray(value[b].T).astype(bf),
            "wqT": wqT, "wkT": wkT, "wvT": wvT, "woT16": woT16,
            "bq_pp": bq_pp, "bk_pp": bk_pp, "bv_r": bv_r, "bo_r": bo_r,
            "ones_r": ones_r,
        })
    return in_maps


def run(in_maps, trace=False, **kw):
    _install_ntff_hook()
    from concourse.bass_utils import run_bass_kernel_spmd
    nc = build_module()
    return run_bass_kernel_spmd(nc, in_maps, core_ids=list(range(N_CORES)),
                                trace=trace, **kw)


def kernel(query, key, value, w_q, b_q, w_k, b_k, w_v, b_v, w_o, b_o):
    in_maps = prepare_in_maps(query, key, value, w_q, b_q, w_k, b_k,
                              w_v, b_v, w_o, b_o)
    res = run(in_maps)
    out = np.stack([res.results[b]["out"].astype(np.float32)
                    for b in range(B)])
    attn_mean = np.stack(
        [res.results[b]["attn_meanT"].T.astype(np.float32)
         for b in range(B)])
    return out, attn_mean
